# revision 13
# baseline (speedup 1.0000x reference)
"""Trainium2 Bass kernel for the ContinuousVariableQNN problem.

Math reduction (validated against the jax reference on host):
  The reference builds a 256x256 symplectic matrix S from params, then
    mu   = mu0 @ S.T   with mu0[:, 0::2] = 2*inputs (odd cols zero)
    n    = (dsum + mu_x^2 + mu_p^2) / (2*hbar) - 0.5
  Because mu0's p-quadrature entries are all zero, the big matmul collapses to
    mu_dev = inputs @ Ms          with Ms[i, j] = S[j, 2*i]   ([128, 256])
  (factor 2 from displacement and the 1/4 normalization cancel), and
    n[b, m] = mu_dev[b, 2m]^2 + mu_dev[b, 2m+1]^2 + bias[m]
  with bias[m] = (diag(S S^T)[2m] + diag(S S^T)[2m+1])/4 - 0.5 (a constant).

Device strategy (pure data parallelism over 8 cores, batch-sharded):
  Per core: 16384 rows. For each 128-row tile:
    PE transpose X tile -> PSUM, DVE copy -> SBUF,
    PE matmul (fp32r)  XT.T @ Ms -> PSUM mu [128, 256],
    ACT Square -> SBUF, DVE pair-add (stride-2), GPSIMD add bias, DMA out.
  DMA layout puts CH consecutive batch rows on one partition so HBM
  transfers use multi-KB descriptors. Input DMAs ride the SP HWDGE queue,
  output DMAs the ACT HWDGE queue.
"""

import numpy as np

import concourse.bass as bass
import concourse.mybir as mybir
import concourse.tile as tile
from concourse import bacc
from concourse.bass_utils import run_bass_kernel_spmd
from concourse.masks import make_identity

N_QUMODES = 128
N_LAYERS = 8
BATCH = 131072
N_CORES = 8
ROWS = BATCH // N_CORES          # 16384 rows per core
CH = 8                           # batch rows per partition per DMA chunk
CHUNK_ROWS = 128 * CH            # 1024
N_CHUNKS = ROWS // CHUNK_ROWS    # 16
SUB = 4                          # tiles (of 128 rows) per compute sub-chunk
F32 = mybir.dt.float32
F32R = mybir.dt.float32r


def host_prep(params: np.ndarray):
    """Build Ms [128, 256] and bias_rep [128, 512] on host (tiny, replicated)."""
    L, N = N_LAYERS, N_QUMODES
    p = params.reshape(L, N, 3).astype(np.float32)
    th1, r, th2 = p[..., 0], p[..., 1], p[..., 2]

    def rot(th):
        c, s = np.cos(th), np.sin(th)
        return np.stack([np.stack([c, -s], -1), np.stack([s, c], -1)], -2)

    z = np.zeros_like(r)
    sq = np.stack([np.stack([np.exp(-r), z], -1),
                   np.stack([z, np.exp(r)], -1)], -2)
    blk = np.einsum('lnab,lnbc,lncd->lnad', rot(th2), sq, rot(th1)).astype(np.float32)

    t = np.float32(np.cos(np.pi / 4))
    rr = np.float32(np.sin(np.pi / 4))
    BS4 = np.array([[t, 0., -rr, 0.],
                    [0., t, 0., -rr],
                    [rr, 0., t, 0.],
                    [0., rr, 0., t]], dtype=np.float32)
    C = np.eye(2 * N, dtype=np.float32)
    for i in range(N - 1):
        C[2 * i:2 * i + 4, :] = BS4 @ C[2 * i:2 * i + 4, :]

    S = np.eye(2 * N, dtype=np.float32)
    idx = np.arange(N)
    for l in range(L):
        D = np.zeros((N, 2, N, 2), np.float32)
        D[idx, :, idx, :] = blk[l]
        S = C @ (D.reshape(2 * N, 2 * N) @ S)

    # Natural interleaved column order: mu[b, 2m] = x_m, mu[b, 2m+1] = p_m.
    Ms = np.ascontiguousarray(S[:, 0::2].T, dtype=np.float32)      # [128, 256]

    dV = (S ** 2).sum(axis=1)                                      # [256]
    bias = ((dV[0::2] + dV[1::2]) / 4.0 - 0.5).astype(np.float32)  # [128]
    bias_rep = np.ascontiguousarray(np.tile(bias, (128, SUB)))     # [128, 512]
    ident = np.eye(128, dtype=np.float32)
    return Ms, bias_rep, ident


def build_bass():
    nc = bacc.Bacc("TRN2", target_bir_lowering=False, debug=False,
                   num_devices=N_CORES)

    x_d = nc.dram_tensor("x", [ROWS, 128], F32R, kind="ExternalInput")
    ms_d = nc.dram_tensor("ms", [128, 256], F32R, kind="ExternalInput")
    bias_d = nc.dram_tensor("bias_rep", [128, SUB * 128], F32, kind="ExternalInput")
    ident_d = nc.dram_tensor("ident", [128, 128], F32R, kind="ExternalInput")
    out_d = nc.dram_tensor("out", [ROWS, 128], F32, kind="ExternalOutput")

    x_v = x_d.ap().rearrange("(c p r) i -> c p r i", p=128, r=CH)
    out_v = out_d.ap().rearrange("(c p r) m -> c p r m", p=128, r=CH)

    with tile.TileContext(nc) as tc:
        with (
            tc.tile_pool(name="const", bufs=1) as const_pool,
            tc.tile_pool(name="xin", bufs=3) as xin_pool,
            tc.tile_pool(name="oout", bufs=3) as oout_pool,
            tc.tile_pool(name="xts", bufs=3) as xts_pool,
            tc.tile_pool(name="sq", bufs=3) as sq_pool,
            tc.tile_pool(name="tmp", bufs=3) as tmp_pool,
            tc.tile_pool(name="xtp", bufs=2, space="PSUM") as xtp_pool,
            tc.tile_pool(name="mup", bufs=3, space="PSUM") as mup_pool,
        ):
            # Get the first input chunk moving before anything else.
            x_first = xin_pool.tile([128, CH, 128], F32R, tag="x_sb")
            nc.sync.dma_start(out=x_first, in_=x_v[0])

            ident = const_pool.tile([128, 128], F32R)
            nc.sync.dma_start(out=ident, in_=ident_d.ap())
            ms_sb = const_pool.tile([128, 256], F32R)
            nc.sync.dma_start(out=ms_sb, in_=ms_d.ap())
            bias_sb = const_pool.tile([128, SUB, 128], F32)
            nc.sync.dma_start(out=bias_sb, in_=bias_d.ap())

            for c in range(N_CHUNKS):
                if c == 0:
                    x_sb = x_first
                else:
                    x_sb = xin_pool.tile([128, CH, 128], F32R, tag="x_sb")
                    nc.sync.dma_start(out=x_sb, in_=x_v[c])
                out_sb = oout_pool.tile([128, CH, 128], F32)

                for s in range(CH // SUB):
                    xt_ps = xtp_pool.tile([128, SUB, 128], F32R)     # 1 bank
                    mu_ps = mup_pool.tile([128, SUB, 256], F32)      # 2 banks
                    xt_sb = xts_pool.tile([128, SUB, 128], F32R)
                    sq_sb = sq_pool.tile([128, SUB, 256], F32)
                    tmp_sb = tmp_pool.tile([128, SUB, 128], F32)

                    for q in range(SUB):
                        nc.tensor.transpose(xt_ps[:, q, :],
                                            x_sb[:, SUB * s + q, :], ident)
                    nc.vector.tensor_copy(xt_sb, xt_ps)
                    for q in range(SUB):
                        nc.tensor.matmul(mu_ps[:, q, :],
                                         xt_sb[:, q, :], ms_sb,
                                         start=True, stop=True)
                    # Square with a de-interleaving AP pair: iterate (q, h, m);
                    # reads walk mu x/p interleaved (stride 2), writes land as
                    # [x-half | p-half] so the pair-add reads contiguous halves.
                    mu_v = mu_ps.rearrange("p a b -> p (a b)").rearrange(
                        "p (q m h) -> p q h m", q=SUB, h=2)
                    sq_flat = sq_sb.rearrange("p a b -> p (a b)")
                    sq_v = sq_flat.rearrange(
                        "p (h q m) -> p q h m", h=2, q=SUB)
                    nc.scalar.activation(sq_v, mu_v,
                                         mybir.ActivationFunctionType.Square)
                    nc.vector.tensor_tensor(out=tmp_sb,
                                            in0=sq_flat[:, 0:SUB * 128],
                                            in1=sq_flat[:, SUB * 128:],
                                            op=mybir.AluOpType.add)
                    nc.gpsimd.tensor_tensor(
                        out=out_sb[:, SUB * s:SUB * (s + 1), :],
                        in0=tmp_sb, in1=bias_sb,
                        op=mybir.AluOpType.add)

                nc.scalar.dma_start(out=out_v[c], in_=out_sb)

    nc.compile()
    return nc


_NC_CACHE = None


def kernel(**inputs: np.ndarray) -> np.ndarray:
    global _NC_CACHE
    X = np.ascontiguousarray(np.asarray(inputs["inputs"], dtype=np.float32))
    params = np.asarray(inputs["params"], dtype=np.float32)
    assert X.shape == (BATCH, N_QUMODES)

    Ms, bias_rep, ident = host_prep(params)

    if _NC_CACHE is None:
        _NC_CACHE = build_bass()
    nc = _NC_CACHE

    in_maps = [
        {"x": X[i * ROWS:(i + 1) * ROWS], "ms": Ms, "bias_rep": bias_rep,
         "ident": ident}
        for i in range(N_CORES)
    ]
    res = run_bass_kernel_spmd(nc, in_maps, core_ids=list(range(N_CORES)))
    out = np.concatenate([r["out"] for r in res.results], axis=0)
    return out.astype(np.float32)


# revision 21
# speedup vs baseline: 1.0948x; 1.0948x over previous
"""Trainium2 Bass kernel for the ContinuousVariableQNN problem.

Math reduction (validated against the jax reference on host):
  The reference builds a 256x256 symplectic matrix S from params, then
    mu   = mu0 @ S.T   with mu0[:, 0::2] = 2*inputs (odd cols zero)
    n    = (dsum + mu_x^2 + mu_p^2) / (2*hbar) - 0.5
  Because mu0's p-quadrature entries are all zero, the big matmul collapses to
    mu_dev = inputs @ Ms          with Ms[i, j] = S[j, 2*i]   ([128, 256])
  (factor 2 from displacement and the 1/4 normalization cancel), and
    n[b, m] = mu_dev[b, 2m]^2 + mu_dev[b, 2m+1]^2 + bias[m]
  with bias[m] = (diag(S S^T)[2m] + diag(S S^T)[2m+1])/4 - 0.5 (a constant).

Device strategy (pure data parallelism over 8 cores, batch-sharded):
  Per core: 16384 rows. For each 128-row tile:
    PE transpose X tile -> PSUM, DVE copy -> SBUF,
    PE matmul (fp32r)  XT.T @ Ms -> PSUM mu [128, 256],
    ACT Square -> SBUF, DVE pair-add (stride-2), GPSIMD add bias, DMA out.
  DMA layout puts CH consecutive batch rows on one partition so HBM
  transfers use multi-KB descriptors. Input DMAs ride the SP HWDGE queue,
  output DMAs the ACT HWDGE queue.
"""

import ml_dtypes
import numpy as np

import concourse.bass as bass
import concourse.mybir as mybir
import concourse.tile as tile
from concourse import bacc
from concourse.bass_utils import run_bass_kernel_spmd
from concourse.masks import make_identity

N_QUMODES = 128
N_LAYERS = 8
BATCH = 131072
N_CORES = 8
ROWS = BATCH // N_CORES          # 16384 rows per core
CH = 8                           # batch rows per partition per DMA chunk
CHUNK_ROWS = 128 * CH            # 1024
N_CHUNKS = ROWS // CHUNK_ROWS    # 16
SUB = 4                          # tiles (of 128 rows) per compute sub-chunk
F32 = mybir.dt.float32
F32R = mybir.dt.float32r
BF16 = mybir.dt.bfloat16


def host_prep(params: np.ndarray):
    """Build Ms [128, 256] and bias_rep [128, 512] on host (tiny, replicated)."""
    L, N = N_LAYERS, N_QUMODES
    p = params.reshape(L, N, 3).astype(np.float32)
    th1, r, th2 = p[..., 0], p[..., 1], p[..., 2]

    def rot(th):
        c, s = np.cos(th), np.sin(th)
        return np.stack([np.stack([c, -s], -1), np.stack([s, c], -1)], -2)

    z = np.zeros_like(r)
    sq = np.stack([np.stack([np.exp(-r), z], -1),
                   np.stack([z, np.exp(r)], -1)], -2)
    blk = np.einsum('lnab,lnbc,lncd->lnad', rot(th2), sq, rot(th1)).astype(np.float32)

    t = np.float32(np.cos(np.pi / 4))
    rr = np.float32(np.sin(np.pi / 4))
    BS4 = np.array([[t, 0., -rr, 0.],
                    [0., t, 0., -rr],
                    [rr, 0., t, 0.],
                    [0., rr, 0., t]], dtype=np.float32)
    C = np.eye(2 * N, dtype=np.float32)
    for i in range(N - 1):
        C[2 * i:2 * i + 4, :] = BS4 @ C[2 * i:2 * i + 4, :]

    S = np.eye(2 * N, dtype=np.float32)
    idx = np.arange(N)
    for l in range(L):
        D = np.zeros((N, 2, N, 2), np.float32)
        D[idx, :, idx, :] = blk[l]
        S = C @ (D.reshape(2 * N, 2 * N) @ S)

    # Natural interleaved column order: mu[b, 2m] = x_m, mu[b, 2m+1] = p_m.
    Ms = np.ascontiguousarray(S[:, 0::2].T, dtype=np.float32)      # [128, 256]

    dV = (S ** 2).sum(axis=1)                                      # [256]
    bias = ((dV[0::2] + dV[1::2]) / 4.0 - 0.5).astype(np.float32)  # [128]
    bias_rep = np.ascontiguousarray(
        np.tile(bias, (128, SUB)).astype(ml_dtypes.bfloat16))      # [128, 512]
    ident = np.eye(128, dtype=np.float32)
    return Ms, bias_rep, ident


def build_bass():
    nc = bacc.Bacc("TRN2", target_bir_lowering=False, debug=False,
                   num_devices=N_CORES)

    x_d = nc.dram_tensor("x", [ROWS, 128], F32R, kind="ExternalInput")
    ms_d = nc.dram_tensor("ms", [128, 256], F32R, kind="ExternalInput")
    bias_d = nc.dram_tensor("bias_rep", [128, SUB * 128], BF16, kind="ExternalInput")
    ident_d = nc.dram_tensor("ident", [128, 128], F32R, kind="ExternalInput")
    out_d = nc.dram_tensor("out", [ROWS, 128], F32, kind="ExternalOutput")

    x_v = x_d.ap().rearrange("(c p r) i -> c p r i", p=128, r=CH)
    out_v = out_d.ap().rearrange("(c p r) m -> c p r m", p=128, r=CH)

    with tile.TileContext(nc) as tc:
        with (
            tc.tile_pool(name="const", bufs=1) as const_pool,
            tc.tile_pool(name="xin", bufs=3) as xin_pool,
            tc.tile_pool(name="oout", bufs=3) as oout_pool,
            tc.tile_pool(name="xts", bufs=3) as xts_pool,
            tc.tile_pool(name="sq", bufs=3) as sq_pool,
            tc.tile_pool(name="tmp", bufs=3) as tmp_pool,
            tc.tile_pool(name="xtp", bufs=2, space="PSUM") as xtp_pool,
            tc.tile_pool(name="mup", bufs=3, space="PSUM") as mup_pool,
        ):
            # Get the first input chunk moving before anything else.
            x_first = xin_pool.tile([128, CH, 128], F32R, tag="x_sb")
            nc.sync.dma_start(out=x_first, in_=x_v[0])

            ident = const_pool.tile([128, 128], F32R)
            nc.sync.dma_start(out=ident, in_=ident_d.ap())
            ms_sb = const_pool.tile([128, 256], F32R)
            nc.sync.dma_start(out=ms_sb, in_=ms_d.ap())
            bias_sb = const_pool.tile([128, SUB, 128], BF16)
            nc.sync.dma_start(out=bias_sb, in_=bias_d.ap())

            for c in range(N_CHUNKS):
                if c == 0:
                    x_sb = x_first
                else:
                    x_sb = xin_pool.tile([128, CH, 128], F32R, tag="x_sb")
                    nc.sync.dma_start(out=x_sb, in_=x_v[c])
                out_sb = oout_pool.tile([128, CH, 128], F32)

                for s in range(CH // SUB):
                    xt_ps = xtp_pool.tile([128, SUB, 128], F32R)     # 1 bank
                    mu_ps = mup_pool.tile([128, SUB, 256], F32)      # 2 banks
                    xt_sb = xts_pool.tile([128, SUB, 128], F32R)
                    sq_sb = sq_pool.tile([128, SUB, 256], BF16)
                    tmp_sb = tmp_pool.tile([128, SUB, 128], BF16)

                    for q in range(SUB):
                        nc.tensor.transpose(xt_ps[:, q, :],
                                            x_sb[:, SUB * s + q, :], ident)
                    nc.vector.tensor_copy(xt_sb, xt_ps)
                    for q in range(SUB):
                        nc.tensor.matmul(mu_ps[:, q, :],
                                         xt_sb[:, q, :], ms_sb,
                                         start=True, stop=True)
                    # Square with a de-interleaving AP pair: iterate (q, h, m);
                    # reads walk mu x/p interleaved (stride 2), writes land as
                    # [x-half | p-half] so the pair-add reads contiguous halves.
                    mu_v = mu_ps.rearrange("p a b -> p (a b)").rearrange(
                        "p (q m h) -> p q h m", q=SUB, h=2)
                    sq_flat = sq_sb.rearrange("p a b -> p (a b)")
                    sq_v = sq_flat.rearrange(
                        "p (h q m) -> p q h m", h=2, q=SUB)
                    nc.scalar.activation(sq_v, mu_v,
                                         mybir.ActivationFunctionType.Square)
                    tmp_flat = tmp_sb.rearrange("p a b -> p (a b)")
                    nc.vector.tensor_tensor(out=tmp_flat,
                                            in0=sq_flat[:, 0:SUB * 128],
                                            in1=sq_flat[:, SUB * 128:],
                                            op=mybir.AluOpType.add)
                    nc.gpsimd.tensor_tensor(
                        out=out_sb[:, SUB * s:SUB * (s + 1), :],
                        in0=tmp_sb, in1=bias_sb,
                        op=mybir.AluOpType.add)

                nc.sync.dma_start(out=out_v[c], in_=out_sb)

    nc.compile()
    return nc


_NC_CACHE = None


def kernel(**inputs: np.ndarray) -> np.ndarray:
    global _NC_CACHE
    X = np.ascontiguousarray(np.asarray(inputs["inputs"], dtype=np.float32))
    params = np.asarray(inputs["params"], dtype=np.float32)
    assert X.shape == (BATCH, N_QUMODES)

    Ms, bias_rep, ident = host_prep(params)

    if _NC_CACHE is None:
        _NC_CACHE = build_bass()
    nc = _NC_CACHE

    in_maps = [
        {"x": X[i * ROWS:(i + 1) * ROWS], "ms": Ms, "bias_rep": bias_rep,
         "ident": ident}
        for i in range(N_CORES)
    ]
    res = run_bass_kernel_spmd(nc, in_maps, core_ids=list(range(N_CORES)))
    out = np.concatenate([r["out"] for r in res.results], axis=0)
    return out.astype(np.float32)
